# revision 12
# baseline (speedup 1.0000x reference)
"""FlyLoRA layer kernel for Trainium2 (8 NeuronCores, data-parallel over tokens).

Computes, for x [4, 4096, 4096], A [32, 4096], B [4096, 32], d [32], k=8:
    y = x @ A.T                      # [B, S, 32]
    mask = top-8 mask of |y + d|     # over the 32 experts
    out = (y * mask) @ B.T * 2.0     # [B, S, 4096]

Sharding: tokens (B*S = 16384) split into 8 contiguous slabs of 2048, one per
core. A/B/d are tiny and replicated. All heavy data is pre-transposed on the
host so every DMA is contiguous per partition.

The kernel is DMA-bound, so x is shipped compactly: an fp16 hi stream plus
(USE_LO) an fp8-e4m3 lo stream holding (x - fp16(x)) * 2^11. A is shipped as
two fp16 limbs (and two scaled e4m3 limbs for the lo stream) packed
side-by-side in the matmul M dimension, so each 128-feature chunk needs one
PE pass per stream. y is reconstructed exactly enough that the top-8
selection matches the f32 reference (0 flipped tokens with USE_LO, ~23
without). The output is stored as fp16 (2.1e-4 rel err) and widened to f32 on
the host.

Per core the 2048 tokens run as 2 halves of 1024, software-pipelined at
emission level: the engines' queues are in-order, so the backend of half h
(recombine, top-k, transposes, mm2, stores) is emitted in slices interleaved
between half h+1's mm1 chunk loop. That keeps the PE continuously fed and
overlaps the back half of one slab with the x streaming of the next.
"""

import os

import numpy as np
import ml_dtypes

import concourse.bacc as bacc
import concourse.tile as tile
from concourse import mybir
from concourse.bass_utils import run_bass_kernel_spmd
from concourse.masks import make_identity

F32 = mybir.dt.float32
F32R = mybir.dt.float32r
F16 = mybir.dt.float16
F8 = mybir.dt.float8e4
ALU = mybir.AluOpType

N_CORES = 8
TOKENS = 16384
TPC = 2048          # tokens per core
D = 4096            # feature dim
R = 32              # experts / lora rank
KC = D // 128       # 32 feature chunks of 128
HALVES = 2
TPH = TPC // HALVES   # 1024 tokens per half
HGROUPS = TPH // 512  # 2 col-packed groups of 512 per half
HCHUNKS = TPH // 128  # 8 token chunks of 128 per half
KPT = 8               # feature chunks per xh tile (tile = 1024 features)
NSLICES = 5           # backend slices per half (front + 4 mm2 slices)

# lo-stream scales: xl = (x - fp16(x)) * 2^XS as e4m3; A limbs scaled by 2^AS
# with the second limb a further 2^AS2 up.
XS = 11
AS = 4
AS2 = 6

USE_LO = bool(int(os.environ.get("KERNEL_USE_LO", "1")))

_nc_cache = {}

# exposed for test.py: last BassKernelResults (for exec_time_ns when tracing)
LAST_RESULT = None


def _build_kernel():
    nc = bacc.Bacc(
        "TRN2",
        target_bir_lowering=False,
        debug=False,
        num_devices=N_CORES,
    )
    xh_d = nc.dram_tensor("xh", [HALVES, D, TPH], F16, kind="ExternalInput").ap()
    xl_d = None
    atpl_d = None
    if USE_LO:
        xl_d = nc.dram_tensor("xl", [HALVES, D, TPH], F8, kind="ExternalInput").ap()
        atpl_d = nc.dram_tensor("ATpl", [128, KC * 2 * R], F8,
                                kind="ExternalInput").ap()
    atph_d = nc.dram_tensor("ATph", [128, KC * 2 * R], F16,
                            kind="ExternalInput").ap()
    bt2_d = nc.dram_tensor("BT2", [R, D], F16, kind="ExternalInput").ap()
    drep_d = nc.dram_tensor("drep", [128, HCHUNKS * R], F32,
                            kind="ExternalInput").ap()
    out_d = nc.dram_tensor("out", [TPC, D], F16, kind="ExternalOutput").ap()

    with tile.TileContext(nc) as tc:
        _kernel_body(tc, out_d, xh_d, xl_d, atph_d, atpl_d, bt2_d, drep_d)
    nc.compile()
    return nc


def _kernel_body(tc, out_d, xh_d, xl_d, atph_d, atpl_d, bt2_d, drep_d):
    nc = tc.nc

    from contextlib import ExitStack

    with ExitStack() as ctx:
        const = ctx.enter_context(tc.tile_pool(name="const", bufs=1))
        work = ctx.enter_context(tc.tile_pool(name="work", bufs=2))
        blk = ctx.enter_context(tc.tile_pool(name="blk", bufs=2))
        xhpool = ctx.enter_context(tc.tile_pool(name="xh", bufs=3))
        xlpool = ctx.enter_context(tc.tile_pool(name="xl", bufs=2)) if USE_LO else None
        ypool = ctx.enter_context(tc.tile_pool(name="ypsum", bufs=2, space="PSUM"))
        tpool = ctx.enter_context(tc.tile_pool(name="tp", bufs=2, space="PSUM"))
        opool = ctx.enter_context(tc.tile_pool(name="opsum", bufs=2, space="PSUM"))
        osb = ctx.enter_context(tc.tile_pool(name="osb", bufs=2))

        # --- constants: weights first (mm1 needs them immediately), B/d
        # later; issued on the otherwise-idle Pool sequencer so the SP can
        # start the x loads at once. ---
        atph_sb = const.tile([128, KC * 2 * R], F16)  # [p, kc*64+32*l+r]
        nc.gpsimd.dma_start(out=atph_sb[:], in_=atph_d[:])
        if USE_LO:
            atpl_sb = const.tile([128, KC * 2 * R], F8)
            nc.gpsimd.dma_start(out=atpl_sb[:], in_=atpl_d[:])
        bt2_sb = const.tile([R, D], F16)          # 2*B^T (fp16 mm2)
        nc.gpsimd.dma_start(out=bt2_sb[:], in_=bt2_d[:])
        drep_sb = const.tile([128, HCHUNKS * R], F32)
        nc.gpsimd.dma_start(out=drep_sb[:], in_=drep_d[:])
        ident = const.tile([128, 128], F32)
        make_identity(nc, ident[:])

        st = [dict() for _ in range(HALVES)]  # per-half live tiles

        def emit_mm1(h, backend_slices):
            """Stream half h's x and accumulate y^T limbs on the PE.

            The PE pays a large penalty on every weight-dtype switch, so the
            fp16 hi sweep (all 32 feature chunks) runs first, then the fp8 lo
            sweep. backend_slices (previous half's backend, itself
            dtype-batched) are interleaved at sweep boundaries."""
            yps = [
                ypool.tile([128, 512], F32, tag=f"yps{g}", name=f"yps{g}")
                for g in range(HGROUPS)
            ]
            st[h]["yps"] = yps
            si = 0
            xl_tiles = []
            for k8 in range(KC // KPT):
                xh_t = xhpool.tile([128, KPT, TPH], F16)
                nc.sync.dma_start(
                    out=xh_t[:],
                    in_=xh_d[h, KPT * 128 * k8:KPT * 128 * (k8 + 1), :]
                    .rearrange("(c p) t -> p c t", p=128),
                )
                if USE_LO and k8 % 2 == 1:
                    xl_t = xlpool.tile([128, 2 * KPT, TPH], F8)
                    nc.sync.dma_start(
                        out=xl_t[:],
                        in_=xl_d[h, KPT * 128 * (k8 - 1):KPT * 128 * (k8 + 1), :]
                        .rearrange("(c p) t -> p c t", p=128),
                    )
                    xl_tiles.append(xl_t)
                for ck in range(KPT):
                    kc = KPT * k8 + ck
                    lhs_hi = atph_sb[:, 2 * R * kc:2 * R * (kc + 1)]
                    for g in range(HGROUPS):
                        nc.tensor.matmul(
                            yps[g][0:2 * R, :],
                            lhs_hi,
                            xh_t[:, ck, 512 * g:512 * (g + 1)],
                            start=(kc == 0),
                            stop=(kc == KC - 1),
                            tile_position=(0, 0),
                        )
                if si < len(backend_slices):
                    backend_slices[si]()
                    si += 1
            if USE_LO:
                for t8, xl_t in enumerate(xl_tiles):
                    for c16 in range(2 * KPT):
                        kc = 2 * KPT * t8 + c16
                        lhs_lo = atpl_sb[:, 2 * R * kc:2 * R * (kc + 1)]
                        for g in range(HGROUPS):
                            nc.tensor.matmul(
                                yps[g][2 * R:4 * R, :],
                                lhs_lo,
                                xl_t[:, c16, 512 * g:512 * (g + 1)],
                                start=(kc == 0),
                                stop=(kc == KC - 1),
                                tile_position=(0, 2 * R),
                            )
                    if si < len(backend_slices):
                        backend_slices[si]()
                        si += 1
            while si < len(backend_slices):
                backend_slices[si]()
                si += 1

        def backend_front(h):
            """Recombine y^T limbs, transpose to token-major, top-8 mask,
            activated y (token-major)."""
            yps = st[h]["yps"]
            yT_sb = work.tile([R, TPH], F32, tag="yT", name="yT")
            for g in range(HGROUPS):
                ysl = yT_sb[:, 512 * g:512 * (g + 1)]
                ha = blk.tile([R, 512], F32, tag="ha", name="ha")
                nc.scalar.copy(ha[:], yps[g][0:R, :])
                if USE_LO:
                    hb = blk.tile([R, 512], F32, tag="hb", name="hb")
                    nc.vector.tensor_add(hb[:], ha[:], yps[g][R:2 * R, :])
                    lc = blk.tile([R, 512], F32, tag="lc", name="lc")
                    nc.scalar.activation(
                        lc[:], yps[g][3 * R:4 * R, :],
                        mybir.ActivationFunctionType.Copy,
                        scale=float(2.0 ** -AS2),
                    )
                    le = blk.tile([R, 512], F32, tag="le", name="le")
                    nc.vector.tensor_add(le[:], lc[:], yps[g][2 * R:3 * R, :])
                    nc.vector.scalar_tensor_tensor(
                        ysl, le[:], float(2.0 ** -(XS + AS)), hb[:],
                        op0=ALU.mult, op1=ALU.add,
                    )
                else:
                    nc.vector.tensor_add(ysl, ha[:], yps[g][R:2 * R, :])

            # transpose y^T -> token-major y [128, 8*32]
            y_sb = work.tile([128, HCHUNKS * R], F32, tag="y", name="y")
            ytok_ps = tpool.tile([128, HCHUNKS * R], F32, tag="tp", name="ytok")
            for c in range(HCHUNKS):
                nc.tensor.transpose(
                    ytok_ps[:, R * c:R * (c + 1)],
                    yT_sb[:, 128 * c:128 * (c + 1)],
                    ident[0:R, 0:R],
                )
            nc.scalar.copy(y_sb[:], ytok_ps[:])

            # top-8 mask of |y + d| per token
            zb = work.tile([128, HCHUNKS * R], F32, tag="zb", name="zb")
            nc.vector.tensor_add(zb[:], y_sb[:], drep_sb[:])
            z = work.tile([128, HCHUNKS * R], F32, tag="z", name="z")
            nc.scalar.activation(z[:], zb[:], mybir.ActivationFunctionType.Abs)
            zap = work.tile([128, HCHUNKS * R], F32, tag="zap", name="zap")
            for c in range(HCHUNKS):
                m8 = blk.tile([128, 8], F32, tag="m8", name="m8")
                zc = z[:, R * c:R * (c + 1)]
                nc.vector.max(out=m8[:], in_=zc)
                nc.vector.match_replace(
                    out=zap[:, R * c:R * (c + 1)],
                    in_to_replace=m8[:],
                    in_values=zc,
                    imm_value=-1.0,
                )
            mask = zb  # reuse
            nc.vector.tensor_scalar(mask[:], zap[:], 0.0, None, op0=ALU.is_lt)
            act = z  # reuse
            nc.vector.tensor_mul(act[:], y_sb[:], mask[:])
            st[h]["act"] = act
            st[h]["actT"] = work.tile([R, TPH], F16, tag="actT", name="actT")

        def backend_mm2(h, s):
            """Slice s (0..3): transpose a 4-chunk group of act (s<2), then
            mm2 + fp16 store for token chunks 2s, 2s+1."""
            act = st[h]["act"]
            actT_sb = st[h]["actT"]
            tok0 = TPH * h
            if s < HCHUNKS // 4:
                pt = tpool.tile([R, 512], F32, tag="tp", name="pt")
                for j in range(4):
                    c = 4 * s + j
                    nc.tensor.transpose(
                        pt[:, 128 * j:128 * (j + 1)],
                        act[:, R * c:R * (c + 1)],
                        ident[:],
                    )
                nc.vector.tensor_copy(actT_sb[:, 512 * s:512 * (s + 1)], pt[:])
            ot = osb.tile([128, 2, D], F16)
            for c2 in range(2):
                c = 2 * s + c2
                lhs = actT_sb[:, 128 * c:128 * (c + 1)]
                for n in range(8):
                    ps = opool.tile([128, 512], F32)
                    nc.tensor.matmul(
                        ps[:],
                        lhs,
                        bt2_sb[:, 512 * n:512 * (n + 1)],
                        start=True,
                        stop=True,
                    )
                    if n % 2 == 0:
                        nc.scalar.copy(ot[:, c2, 512 * n:512 * (n + 1)], ps[:])
                    else:
                        nc.vector.tensor_copy(ot[:, c2, 512 * n:512 * (n + 1)], ps[:])
            row0 = tok0 + 256 * s
            nc.gpsimd.dma_start(
                out=out_d[row0:row0 + 256, :].rearrange("(c p) n -> p c n", p=128),
                in_=ot[:],
            )

        def backend_slices(h):
            sl = [lambda h=h: backend_front(h)]
            for s in range(4):
                sl.append(lambda h=h, s=s: backend_mm2(h, s))
            return sl

        for h in range(HALVES):
            emit_mm1(h, backend_slices(h - 1) if h > 0 else [])
        for fn in backend_slices(HALVES - 1):
            fn()


def _get_nc():
    if "nc" not in _nc_cache:
        _nc_cache["nc"] = _build_kernel()
    return _nc_cache["nc"]


def _pack_a_limbs(hi, lo):
    """[2 limbs, R, D] -> [128, KC*2R] with [p, kc*64+32*l+r] = limb_l[r, 128kc+p]."""
    both = np.stack([hi, lo], axis=0)              # [l, r, D]
    tmp = both.reshape(2, R, KC, 128)              # [l, r, kc, p]
    return np.ascontiguousarray(
        tmp.transpose(3, 2, 0, 1).reshape(128, KC * 2 * R)
    )


def kernel(x, A, B, d, k):
    global LAST_RESULT
    assert int(k) == 8, f"kernel hardcodes k=8, got {k}"
    x = np.asarray(x, dtype=np.float32)
    A = np.asarray(A, dtype=np.float32)
    B = np.asarray(B, dtype=np.float32)
    d = np.asarray(d, dtype=np.float32)
    assert x.shape == (4, 4096, 4096) and A.shape == (R, D) and B.shape == (D, R)

    X = x.reshape(TOKENS, D)
    xh16 = X.astype(np.float16)
    xhT = xh16.T                                   # [D, TOKENS] view
    if USE_LO:
        xl8 = ((X - xh16.astype(np.float32)) * np.float32(2.0 ** XS)).astype(
            ml_dtypes.float8_e4m3)
        xlT = xl8.T

    Ah = A.astype(np.float16)
    Al = (A - Ah.astype(np.float32)).astype(np.float16)
    ATph = _pack_a_limbs(Ah, Al)
    if USE_LO:
        As = A * np.float32(2.0 ** AS)
        A8h = As.astype(ml_dtypes.float8_e4m3)
        A8l = ((As - A8h.astype(np.float32)) * np.float32(2.0 ** AS2)).astype(
            ml_dtypes.float8_e4m3)
        ATpl = _pack_a_limbs(A8h, A8l)

    BT2 = (np.ascontiguousarray(B.T) * np.float32(2.0)).astype(np.float16)
    drep = np.ascontiguousarray(np.tile(d, (128, HCHUNKS)))           # [128, 256]

    nc = _get_nc()
    in_maps = []
    for c in range(N_CORES):
        xhh = np.stack([
            np.ascontiguousarray(
                xhT[:, c * TPC + hh * TPH: c * TPC + (hh + 1) * TPH])
            for hh in range(HALVES)
        ])
        m = {
            "xh": xhh,
            "ATph": ATph,
            "BT2": BT2,
            "drep": drep,
        }
        if USE_LO:
            m["xl"] = np.stack([
                np.ascontiguousarray(
                    xlT[:, c * TPC + hh * TPH: c * TPC + (hh + 1) * TPH])
                for hh in range(HALVES)
            ])
            m["ATpl"] = ATpl
        in_maps.append(m)
    trace = bool(int(os.environ.get("KERNEL_TRACE", "0")))
    res = run_bass_kernel_spmd(nc, in_maps, list(range(N_CORES)), trace=trace)
    LAST_RESULT = res
    outs = [res.results[c]["out"] for c in range(N_CORES)]
    full = np.concatenate(outs, axis=0).astype(np.float32)            # [16384, 4096]
    return full.reshape(4, 4096, 4096)


# revision 14
# speedup vs baseline: 1.4762x; 1.4762x over previous
"""FlyLoRA layer kernel for Trainium2 (8 NeuronCores, data-parallel over tokens).

Computes, for x [4, 4096, 4096], A [32, 4096], B [4096, 32], d [32], k=8:
    y = x @ A.T                      # [B, S, 32]
    mask = top-8 mask of |y + d|     # over the 32 experts
    out = (y * mask) @ B.T * 2.0     # [B, S, 4096]

Sharding: tokens (B*S = 16384) split into 8 contiguous slabs of 2048, one per
core. A/B/d are tiny and replicated. All heavy data is pre-transposed on the
host so every DMA is contiguous per partition.

The kernel is DMA-bound, so x is shipped compactly: an fp16 hi stream plus
(USE_LO) an fp8-e4m3 lo stream holding (x - fp16(x)) * 2^11. A is shipped as
two fp16 limbs (and two scaled e4m3 limbs for the lo stream) packed
side-by-side in the matmul M dimension. y is reconstructed exactly enough
that the top-8 selection matches the f32 reference (0 flipped tokens with
USE_LO, ~23 of 16384 without). mm2 runs in fp16 (value error only); the
output is stored as fp16 and widened to f32 on the host.

The PE clock ramps (1.2 -> 2.4 GHz) only after ~3.5us of continuous
execution, so the emission order is arranged to give the PE few, long,
continuous blocks: the 2048 tokens run as 4 quarters of 512, and the PE
stream per window is [mm1(q) | y-transpose(q) | act-transpose+mm2(q-1)],
where each segment's cross-engine dependencies (recombine, top-k on DVE/ACT)
were produced during the previous PE segment. mm1 alternates PSUM banks by
feature-chunk parity so back-to-back accumulation never stalls the array.
"""

import os

import numpy as np
import ml_dtypes

import concourse.bacc as bacc
import concourse.tile as tile
from concourse import mybir
from concourse.bass_utils import run_bass_kernel_spmd
from concourse.masks import make_identity

F32 = mybir.dt.float32
F16 = mybir.dt.float16
F8 = mybir.dt.float8e4
ALU = mybir.AluOpType

N_CORES = 8
TOKENS = 16384
TPC = 2048          # tokens per core
D = 4096            # feature dim
R = 32              # experts / lora rank
KC = D // 128       # 32 feature chunks of 128
QUARTERS = 4
TPQ = TPC // QUARTERS   # 512 tokens per quarter
QCHUNKS = TPQ // 128    # 4 token chunks of 128 per quarter
KPT = 8                 # feature chunks per xh tile (tile = 1024 features)

# lo-stream scales: xl = (x - fp16(x)) * 2^XS as e4m3; A limbs scaled by 2^AS
# with the second limb a further 2^AS2 up.
XS = 11
AS = 4
AS2 = 6

USE_LO = bool(int(os.environ.get("KERNEL_USE_LO", "0")))

_nc_cache = {}

# exposed for test.py: last BassKernelResults (for exec_time_ns when tracing)
LAST_RESULT = None


def _build_kernel():
    nc = bacc.Bacc(
        "TRN2",
        target_bir_lowering=False,
        debug=False,
        num_devices=N_CORES,
    )
    xh_d = nc.dram_tensor("xh", [QUARTERS, D, TPQ], F16, kind="ExternalInput").ap()
    xl_d = None
    atpl_d = None
    if USE_LO:
        xl_d = nc.dram_tensor("xl", [QUARTERS, D, TPQ], F8,
                              kind="ExternalInput").ap()
        atpl_d = nc.dram_tensor("ATpl", [128, KC * 2 * R], F8,
                                kind="ExternalInput").ap()
    atph_d = nc.dram_tensor("ATph", [128, KC * 2 * R], F16,
                            kind="ExternalInput").ap()
    bt2_d = nc.dram_tensor("BT2", [R, D], F16, kind="ExternalInput").ap()
    drep_d = nc.dram_tensor("drep", [128, QCHUNKS * R], F32,
                            kind="ExternalInput").ap()
    out_d = nc.dram_tensor("out", [TPC, D], F16, kind="ExternalOutput").ap()

    with tile.TileContext(nc) as tc:
        _kernel_body(tc, out_d, xh_d, xl_d, atph_d, atpl_d, bt2_d, drep_d)
    nc.compile()
    return nc


def _kernel_body(tc, out_d, xh_d, xl_d, atph_d, atpl_d, bt2_d, drep_d):
    nc = tc.nc

    from contextlib import ExitStack

    with ExitStack() as ctx:
        const = ctx.enter_context(tc.tile_pool(name="const", bufs=1))
        work = ctx.enter_context(tc.tile_pool(name="work", bufs=2))
        blk = ctx.enter_context(tc.tile_pool(name="blk", bufs=2))
        xhpool = ctx.enter_context(tc.tile_pool(name="xh", bufs=6))
        xlpool = ctx.enter_context(tc.tile_pool(name="xl", bufs=3)) if USE_LO else None
        ypool = ctx.enter_context(tc.tile_pool(name="ypsum", bufs=2, space="PSUM"))
        tpool = ctx.enter_context(tc.tile_pool(name="tp", bufs=2, space="PSUM"))
        opool = ctx.enter_context(tc.tile_pool(name="opsum", bufs=2, space="PSUM"))
        osb = ctx.enter_context(tc.tile_pool(name="osb", bufs=3))

        # --- constants: weights first (mm1 needs them immediately); issued on
        # the otherwise-idle Pool sequencer so the SP starts x loads at once.
        atph_sb = const.tile([128, KC * 2 * R], F16)  # [p, kc*64+32*l+r]
        nc.gpsimd.dma_start(out=atph_sb[:], in_=atph_d[:])
        if USE_LO:
            atpl_sb = const.tile([128, KC * 2 * R], F8)
            nc.gpsimd.dma_start(out=atpl_sb[:], in_=atpl_d[:])
        bt2_sb = const.tile([R, D], F16)          # 2*B^T (fp16 mm2)
        nc.gpsimd.dma_start(out=bt2_sb[:], in_=bt2_d[:])
        drep_sb = const.tile([128, QCHUNKS * R], F32)
        nc.gpsimd.dma_start(out=drep_sb[:], in_=drep_d[:])
        ident = const.tile([128, 128], F32)
        make_identity(nc, ident[:])

        st = [dict() for _ in range(QUARTERS)]  # per-quarter live tiles

        def emit_mm1(q):
            """Stream quarter q's x; accumulate y^T limbs on the PE.
            Feature chunks alternate between two PSUM banks (parity) so
            back-to-back accumulation never serializes on one bank. Bank
            partitions: 0-63 hi limbs (Ah|Al), 64-127 lo limbs."""
            yps = [
                ypool.tile([128, TPQ], F32, tag=f"yps{p}", name=f"yps{p}")
                for p in range(2)
            ]
            st[q]["yps"] = yps
            xl_tiles = []
            for k8 in range(KC // KPT):
                xh_t = xhpool.tile([128, KPT, TPQ], F16)
                nc.sync.dma_start(
                    out=xh_t[:],
                    in_=xh_d[q, KPT * 128 * k8:KPT * 128 * (k8 + 1), :]
                    .rearrange("(c p) t -> p c t", p=128),
                )
                if USE_LO and k8 % 2 == 1:
                    xl_t = xlpool.tile([128, 2 * KPT, TPQ], F8)
                    nc.sync.dma_start(
                        out=xl_t[:],
                        in_=xl_d[q, KPT * 128 * (k8 - 1):KPT * 128 * (k8 + 1), :]
                        .rearrange("(c p) t -> p c t", p=128),
                    )
                    xl_tiles.append(xl_t)
                for ck in range(KPT):
                    kc = KPT * k8 + ck
                    nc.tensor.matmul(
                        yps[kc % 2][0:2 * R, :],
                        atph_sb[:, 2 * R * kc:2 * R * (kc + 1)],
                        xh_t[:, ck, :],
                        start=(kc < 2),
                        stop=(kc >= KC - 2),
                        tile_position=(0, 0),
                    )
            if USE_LO:
                for t8, xl_t in enumerate(xl_tiles):
                    for c16 in range(2 * KPT):
                        kc = 2 * KPT * t8 + c16
                        nc.tensor.matmul(
                            yps[kc % 2][2 * R:4 * R, :],
                            atpl_sb[:, 2 * R * kc:2 * R * (kc + 1)],
                            xl_t[:, c16, :],
                            start=(kc < 2),
                            stop=(kc >= KC - 2),
                            tile_position=(0, 2 * R),
                        )

        def emit_front_alu(q):
            """DVE/ACT: recombine y^T limbs from the two parity banks.
            Each ALU op may read at most one PSUM operand."""
            yps = st[q]["yps"]
            yT_sb = work.tile([R, TPQ], F32, tag="yT", name="yT")
            st[q]["yT"] = yT_sb
            ha = blk.tile([R, TPQ], F32, tag="ha", name="ha")
            nc.scalar.copy(ha[:], yps[0][0:R, :])
            hb = blk.tile([R, TPQ], F32, tag="hb", name="hb")
            nc.vector.tensor_add(hb[:], ha[:], yps[0][R:2 * R, :])
            hc = blk.tile([R, TPQ], F32, tag="hc", name="hc")
            nc.scalar.copy(hc[:], yps[1][0:R, :])
            hd = blk.tile([R, TPQ], F32, tag="hd", name="hd")
            nc.vector.tensor_add(hd[:], hc[:], yps[1][R:2 * R, :])
            if USE_LO:
                hi = blk.tile([R, TPQ], F32, tag="hi", name="hi")
                nc.vector.tensor_add(hi[:], hb[:], hd[:])
                lc0 = blk.tile([R, TPQ], F32, tag="lc0", name="lc0")
                nc.scalar.activation(
                    lc0[:], yps[0][3 * R:4 * R, :],
                    mybir.ActivationFunctionType.Copy, scale=float(2.0 ** -AS2))
                le0 = blk.tile([R, TPQ], F32, tag="le0", name="le0")
                nc.vector.tensor_add(le0[:], lc0[:], yps[0][2 * R:3 * R, :])
                lc1 = blk.tile([R, TPQ], F32, tag="lc1", name="lc1")
                nc.scalar.activation(
                    lc1[:], yps[1][3 * R:4 * R, :],
                    mybir.ActivationFunctionType.Copy, scale=float(2.0 ** -AS2))
                le1 = blk.tile([R, TPQ], F32, tag="le1", name="le1")
                nc.vector.tensor_add(le1[:], lc1[:], yps[1][2 * R:3 * R, :])
                lo = blk.tile([R, TPQ], F32, tag="lo", name="lo")
                nc.vector.tensor_add(lo[:], le0[:], le1[:])
                nc.vector.scalar_tensor_tensor(
                    yT_sb[:], lo[:], float(2.0 ** -(XS + AS)), hi[:],
                    op0=ALU.mult, op1=ALU.add,
                )
            else:
                nc.vector.tensor_add(yT_sb[:], hb[:], hd[:])

        def emit_ytok_trans(q):
            """PE: transpose y^T -> token-major [128, QCHUNKS*R] (needs
            recombine(q) done on DVE)."""
            yT_sb = st[q]["yT"]
            ytok_ps = tpool.tile([128, QCHUNKS * R], F32, tag="tp", name="ytok")
            st[q]["ytok"] = ytok_ps
            for c in range(QCHUNKS):
                nc.tensor.transpose(
                    ytok_ps[:, R * c:R * (c + 1)],
                    yT_sb[:, 128 * c:128 * (c + 1)],
                    ident[0:R, 0:R],
                )

        def emit_topk(q):
            """ACT/DVE: top-8 mask of |y + d| per token, activated y."""
            ytok_ps = st[q]["ytok"]
            y_sb = work.tile([128, QCHUNKS * R], F32, tag="y", name="y")
            nc.scalar.copy(y_sb[:], ytok_ps[:])
            zb = work.tile([128, QCHUNKS * R], F32, tag="zb", name="zb")
            nc.vector.tensor_add(zb[:], y_sb[:], drep_sb[:])
            z = work.tile([128, QCHUNKS * R], F32, tag="z", name="z")
            nc.scalar.activation(z[:], zb[:], mybir.ActivationFunctionType.Abs)
            zap = work.tile([128, QCHUNKS * R], F32, tag="zap", name="zap")
            for c in range(QCHUNKS):
                m8 = blk.tile([128, 8], F32, tag="m8", name="m8")
                zc = z[:, R * c:R * (c + 1)]
                nc.vector.max(out=m8[:], in_=zc)
                nc.vector.match_replace(
                    out=zap[:, R * c:R * (c + 1)],
                    in_to_replace=m8[:],
                    in_values=zc,
                    imm_value=-1.0,
                )
            mask = zb  # reuse
            nc.vector.tensor_scalar(mask[:], zap[:], 0.0, None, op0=ALU.is_lt)
            act = z  # reuse
            nc.vector.tensor_mul(act[:], y_sb[:], mask[:])
            st[q]["act"] = act

        def emit_backend_pe(q):
            """PE: transpose act to expert-major, fp16 mm2, fp16 stores
            (needs topk(q), which ran during the following quarter's mm1)."""
            act = st[q]["act"]
            pt = tpool.tile([R, TPQ], F32, tag="tp", name="pt")
            for c in range(QCHUNKS):
                nc.tensor.transpose(
                    pt[:, 128 * c:128 * (c + 1)],
                    act[:, R * c:R * (c + 1)],
                    ident[:],
                )
            actT_sb = work.tile([R, TPQ], F16, tag="actT", name="actT")
            nc.vector.tensor_copy(actT_sb[:], pt[:])
            tok0 = TPQ * q
            for half in range(2):
                ot = osb.tile([128, 2, D], F16)
                for c2 in range(2):
                    c = 2 * half + c2
                    lhs = actT_sb[:, 128 * c:128 * (c + 1)]
                    for n in range(8):
                        ps = opool.tile([128, 512], F32)
                        nc.tensor.matmul(
                            ps[:],
                            lhs,
                            bt2_sb[:, 512 * n:512 * (n + 1)],
                            start=True,
                            stop=True,
                        )
                        if n % 2 == 0:
                            nc.scalar.copy(ot[:, c2, 512 * n:512 * (n + 1)], ps[:])
                        else:
                            nc.vector.tensor_copy(
                                ot[:, c2, 512 * n:512 * (n + 1)], ps[:])
                row0 = tok0 + 256 * half
                nc.gpsimd.dma_start(
                    out=out_d[row0:row0 + 256, :]
                    .rearrange("(c p) n -> p c n", p=128),
                    in_=ot[:],
                )

        # --- software-pipelined emission: PE stream per window is
        # [mm1(q) | ytok(q) | act-trans+mm2(q-1)]; DVE/ACT fill in
        # recombine(q) right after mm1(q) and topk(q) during mm1(q+1). ---
        for q in range(QUARTERS):
            emit_mm1(q)
            emit_front_alu(q)
            if q > 0:
                emit_backend_pe(q - 1)
            emit_ytok_trans(q)
            emit_topk(q)
        emit_backend_pe(QUARTERS - 1)


def _get_nc():
    if "nc" not in _nc_cache:
        _nc_cache["nc"] = _build_kernel()
    return _nc_cache["nc"]


def _pack_a_limbs(hi, lo):
    """[2 limbs, R, D] -> [128, KC*2R] with [p, kc*64+32*l+r] = limb_l[r, 128kc+p]."""
    both = np.stack([hi, lo], axis=0)              # [l, r, D]
    tmp = both.reshape(2, R, KC, 128)              # [l, r, kc, p]
    return np.ascontiguousarray(
        tmp.transpose(3, 2, 0, 1).reshape(128, KC * 2 * R)
    )


def kernel(x, A, B, d, k):
    global LAST_RESULT
    assert int(k) == 8, f"kernel hardcodes k=8, got {k}"
    x = np.asarray(x, dtype=np.float32)
    A = np.asarray(A, dtype=np.float32)
    B = np.asarray(B, dtype=np.float32)
    d = np.asarray(d, dtype=np.float32)
    assert x.shape == (4, 4096, 4096) and A.shape == (R, D) and B.shape == (D, R)

    X = x.reshape(TOKENS, D)
    xh16 = X.astype(np.float16)
    xhT = xh16.T                                   # [D, TOKENS] view
    if USE_LO:
        xl8 = ((X - xh16.astype(np.float32)) * np.float32(2.0 ** XS)).astype(
            ml_dtypes.float8_e4m3)
        xlT = xl8.T

    Ah = A.astype(np.float16)
    Al = (A - Ah.astype(np.float32)).astype(np.float16)
    ATph = _pack_a_limbs(Ah, Al)
    if USE_LO:
        As = A * np.float32(2.0 ** AS)
        A8h = As.astype(ml_dtypes.float8_e4m3)
        A8l = ((As - A8h.astype(np.float32)) * np.float32(2.0 ** AS2)).astype(
            ml_dtypes.float8_e4m3)
        ATpl = _pack_a_limbs(A8h, A8l)

    BT2 = (np.ascontiguousarray(B.T) * np.float32(2.0)).astype(np.float16)
    drep = np.ascontiguousarray(np.tile(d, (128, QCHUNKS)))           # [128, 128]

    nc = _get_nc()
    in_maps = []
    for c in range(N_CORES):
        xhh = np.stack([
            np.ascontiguousarray(
                xhT[:, c * TPC + qq * TPQ: c * TPC + (qq + 1) * TPQ])
            for qq in range(QUARTERS)
        ])
        m = {
            "xh": xhh,
            "ATph": ATph,
            "BT2": BT2,
            "drep": drep,
        }
        if USE_LO:
            m["xl"] = np.stack([
                np.ascontiguousarray(
                    xlT[:, c * TPC + qq * TPQ: c * TPC + (qq + 1) * TPQ])
                for qq in range(QUARTERS)
            ])
            m["ATpl"] = ATpl
        in_maps.append(m)
    trace = bool(int(os.environ.get("KERNEL_TRACE", "0")))
    res = run_bass_kernel_spmd(nc, in_maps, list(range(N_CORES)), trace=trace)
    LAST_RESULT = res
    outs = [res.results[c]["out"] for c in range(N_CORES)]
    full = np.concatenate(outs, axis=0).astype(np.float32)            # [16384, 4096]
    return full.reshape(4, 4096, 4096)


# revision 18
# speedup vs baseline: 1.6031x; 1.0860x over previous
"""FlyLoRA layer kernel for Trainium2 (8 NeuronCores, data-parallel over tokens).

Computes, for x [4, 4096, 4096], A [32, 4096], B [4096, 32], d [32], k=8:
    y = x @ A.T                      # [B, S, 32]
    mask = top-8 mask of |y + d|     # over the 32 experts
    out = (y * mask) @ B.T * 2.0     # [B, S, 4096]

Sharding: tokens (B*S = 16384) split into 8 contiguous slabs of 2048, one per
core. A/B/d are tiny and replicated. All heavy data is pre-transposed on the
host so every DMA is contiguous per partition.

The kernel is DMA-bound, so x is shipped compactly: an fp16 hi stream plus
(USE_LO) an fp8-e4m3 lo stream holding (x - fp16(x)) * 2^11. A is shipped as
two fp16 limbs (and two scaled e4m3 limbs for the lo stream) packed
side-by-side in the matmul M dimension. y is reconstructed exactly enough
that the top-8 selection matches the f32 reference (0 flipped tokens with
USE_LO, ~23 of 16384 without). mm2 runs in fp16 (value error only); the
output is stored as fp16 and widened to f32 on the host.

The PE clock ramps (1.2 -> 2.4 GHz) only after ~3.5us of continuous
execution, so the emission order is arranged to give the PE few, long,
continuous blocks: the 2048 tokens run as 4 quarters of 512, and the PE
stream per window is [mm1(q) | y-transpose(q) | act-transpose+mm2(q-1)],
where each segment's cross-engine dependencies (recombine, top-k on DVE/ACT)
were produced during the previous PE segment. mm1 alternates PSUM banks by
feature-chunk parity so back-to-back accumulation never stalls the array.
"""

import os

import numpy as np
import ml_dtypes

import concourse.bacc as bacc
import concourse.tile as tile
from concourse import mybir
from concourse.bass_utils import run_bass_kernel_spmd
from concourse.masks import make_identity

F32 = mybir.dt.float32
F16 = mybir.dt.float16
F8 = mybir.dt.float8e4
ALU = mybir.AluOpType

N_CORES = 8
TOKENS = 16384
TPC = 2048          # tokens per core
D = 4096            # feature dim
R = 32              # experts / lora rank
KC = D // 128       # 32 feature chunks of 128
QUARTERS = 4
TPQ = TPC // QUARTERS   # 512 tokens per quarter
QCHUNKS = TPQ // 128    # 4 token chunks of 128 per quarter
KPT = 8                 # feature chunks per xh tile (tile = 1024 features)

# lo-stream scales: xl = (x - fp16(x)) * 2^XS as e4m3; A limbs scaled by 2^AS
# with the second limb a further 2^AS2 up.
XS = 11
AS = 4
AS2 = 6

USE_LO = bool(int(os.environ.get("KERNEL_USE_LO", "0")))

_nc_cache = {}

# exposed for test.py: last BassKernelResults (for exec_time_ns when tracing)
LAST_RESULT = None


def _build_kernel():
    nc = bacc.Bacc(
        "TRN2",
        target_bir_lowering=False,
        debug=False,
        num_devices=N_CORES,
    )
    xh_d = nc.dram_tensor("xh", [QUARTERS, D, TPQ], F16, kind="ExternalInput").ap()
    xl_d = None
    atpl_d = None
    if USE_LO:
        xl_d = nc.dram_tensor("xl", [QUARTERS, D, TPQ], F8,
                              kind="ExternalInput").ap()
        atpl_d = nc.dram_tensor("ATpl", [128, KC * 2 * R], F8,
                                kind="ExternalInput").ap()
    atph_d = nc.dram_tensor("ATph", [128, KC * 2 * R], F16,
                            kind="ExternalInput").ap()
    bt2_d = nc.dram_tensor("BT2", [R, D], F16, kind="ExternalInput").ap()
    drep_d = nc.dram_tensor("drep", [128, QCHUNKS * R], F32,
                            kind="ExternalInput").ap()
    out_d = nc.dram_tensor("out", [TPC, D], F16, kind="ExternalOutput").ap()

    with tile.TileContext(nc) as tc:
        _kernel_body(tc, out_d, xh_d, xl_d, atph_d, atpl_d, bt2_d, drep_d)
    nc.compile()
    return nc


def _kernel_body(tc, out_d, xh_d, xl_d, atph_d, atpl_d, bt2_d, drep_d):
    nc = tc.nc

    from contextlib import ExitStack

    with ExitStack() as ctx:
        const = ctx.enter_context(tc.tile_pool(name="const", bufs=1))
        work = ctx.enter_context(tc.tile_pool(name="work", bufs=2))
        blk = ctx.enter_context(tc.tile_pool(name="blk", bufs=2))
        xhpool = ctx.enter_context(tc.tile_pool(name="xh", bufs=6))
        xlpool = ctx.enter_context(tc.tile_pool(name="xl", bufs=3)) if USE_LO else None
        ypool = ctx.enter_context(tc.tile_pool(name="ypsum", bufs=1, space="PSUM"))
        tpool = ctx.enter_context(tc.tile_pool(name="tp", bufs=2, space="PSUM"))
        opool = ctx.enter_context(tc.tile_pool(name="opsum", bufs=4, space="PSUM"))
        osb = ctx.enter_context(tc.tile_pool(name="osb", bufs=3))

        # --- constants: weights first (mm1 needs them immediately); issued on
        # the otherwise-idle Pool sequencer so the SP starts x loads at once.
        atph_sb = const.tile([128, KC * 2 * R], F16)  # [p, kc*64+32*l+r]
        nc.gpsimd.dma_start(out=atph_sb[:], in_=atph_d[:])
        if USE_LO:
            atpl_sb = const.tile([128, KC * 2 * R], F8)
            nc.gpsimd.dma_start(out=atpl_sb[:], in_=atpl_d[:])
        bt2_sb = const.tile([R, D], F16)          # 2*B^T (fp16 mm2)
        nc.gpsimd.dma_start(out=bt2_sb[:], in_=bt2_d[:])
        drep_sb = const.tile([128, QCHUNKS * R], F32)
        nc.gpsimd.dma_start(out=drep_sb[:], in_=drep_d[:])
        ident = const.tile([128, 128], F32)
        make_identity(nc, ident[:])

        st = [dict() for _ in range(QUARTERS)]  # per-quarter live tiles

        def emit_mm1(q):
            """Stream quarter q's x; accumulate y^T limbs on the PE.
            Feature chunks alternate between two PSUM banks (parity) so
            back-to-back accumulation never serializes on one bank. Bank
            partitions: 0-63 hi limbs (Ah|Al), 64-127 lo limbs."""
            yps = [
                ypool.tile([128, TPQ], F32, tag=f"yps{p}", name=f"yps{p}")
                for p in range(2)
            ]
            st[q]["yps"] = yps
            xl_tiles = []
            # q0 starts with small tiles so the first matmul issues early
            sizes = [2, 6, 8, 8, 8] if q == 0 else [8, 8, 8, 8]
            kc0 = 0
            xl_issued = 0
            for kn in sizes:
                xh_t = xhpool.tile([128, kn, TPQ], F16, tag="xh", name="xh")
                nc.sync.dma_start(
                    out=xh_t[:],
                    in_=xh_d[q, 128 * kc0:128 * (kc0 + kn), :]
                    .rearrange("(c p) t -> p c t", p=128),
                )
                if USE_LO and kc0 + kn >= 2 * KPT * (xl_issued + 1):
                    xl_t = xlpool.tile([128, 2 * KPT, TPQ], F8)
                    nc.sync.dma_start(
                        out=xl_t[:],
                        in_=xl_d[q, 2 * KPT * 128 * xl_issued:
                                 2 * KPT * 128 * (xl_issued + 1), :]
                        .rearrange("(c p) t -> p c t", p=128),
                    )
                    xl_tiles.append(xl_t)
                    xl_issued += 1
                for ck in range(kn):
                    kc = kc0 + ck
                    nc.tensor.matmul(
                        yps[kc % 2][0:2 * R, :],
                        atph_sb[:, 2 * R * kc:2 * R * (kc + 1)],
                        xh_t[:, ck, :],
                        start=(kc < 2),
                        stop=(kc >= KC - 2),
                        tile_position=(0, 0),
                    )
                kc0 += kn
            if USE_LO:
                for t8, xl_t in enumerate(xl_tiles):
                    for c16 in range(2 * KPT):
                        kc = 2 * KPT * t8 + c16
                        nc.tensor.matmul(
                            yps[kc % 2][2 * R:4 * R, :],
                            atpl_sb[:, 2 * R * kc:2 * R * (kc + 1)],
                            xl_t[:, c16, :],
                            start=(kc < 2),
                            stop=(kc >= KC - 2),
                            tile_position=(0, 2 * R),
                        )

        def emit_front_alu(q):
            """DVE/ACT: recombine y^T limbs from the two parity banks.
            Each ALU op may read at most one PSUM operand."""
            yps = st[q]["yps"]
            yT_sb = work.tile([R, TPQ], F32, tag="yT", name="yT")
            st[q]["yT"] = yT_sb
            ha = blk.tile([R, TPQ], F32, tag="ha", name="ha")
            nc.scalar.copy(ha[:], yps[0][0:R, :])
            hb = blk.tile([R, TPQ], F32, tag="hb", name="hb")
            nc.vector.tensor_add(hb[:], ha[:], yps[0][R:2 * R, :])
            hc = blk.tile([R, TPQ], F32, tag="hc", name="hc")
            nc.scalar.copy(hc[:], yps[1][0:R, :])
            hd = blk.tile([R, TPQ], F32, tag="hd", name="hd")
            nc.vector.tensor_add(hd[:], hc[:], yps[1][R:2 * R, :])
            if USE_LO:
                hi = blk.tile([R, TPQ], F32, tag="hi", name="hi")
                nc.vector.tensor_add(hi[:], hb[:], hd[:])
                lc0 = blk.tile([R, TPQ], F32, tag="lc0", name="lc0")
                nc.scalar.activation(
                    lc0[:], yps[0][3 * R:4 * R, :],
                    mybir.ActivationFunctionType.Copy, scale=float(2.0 ** -AS2))
                le0 = blk.tile([R, TPQ], F32, tag="le0", name="le0")
                nc.vector.tensor_add(le0[:], lc0[:], yps[0][2 * R:3 * R, :])
                lc1 = blk.tile([R, TPQ], F32, tag="lc1", name="lc1")
                nc.scalar.activation(
                    lc1[:], yps[1][3 * R:4 * R, :],
                    mybir.ActivationFunctionType.Copy, scale=float(2.0 ** -AS2))
                le1 = blk.tile([R, TPQ], F32, tag="le1", name="le1")
                nc.vector.tensor_add(le1[:], lc1[:], yps[1][2 * R:3 * R, :])
                lo = blk.tile([R, TPQ], F32, tag="lo", name="lo")
                nc.vector.tensor_add(lo[:], le0[:], le1[:])
                nc.vector.scalar_tensor_tensor(
                    yT_sb[:], lo[:], float(2.0 ** -(XS + AS)), hi[:],
                    op0=ALU.mult, op1=ALU.add,
                )
            else:
                nc.vector.tensor_add(yT_sb[:], hb[:], hd[:])

        def emit_ytok_trans(q):
            """PE: transpose y^T -> token-major [128, QCHUNKS*R] (needs
            recombine(q) done on DVE)."""
            yT_sb = st[q]["yT"]
            ytok_ps = tpool.tile([128, QCHUNKS * R], F32, tag="tp", name="ytok")
            st[q]["ytok"] = ytok_ps
            for c in range(QCHUNKS):
                nc.tensor.transpose(
                    ytok_ps[:, R * c:R * (c + 1)],
                    yT_sb[:, 128 * c:128 * (c + 1)],
                    ident[0:R, 0:R],
                )

        def emit_topk(q):
            """ACT/DVE: top-8 mask of |y + d| per token, activated y."""
            ytok_ps = st[q]["ytok"]
            y_sb = work.tile([128, QCHUNKS * R], F32, tag="y", name="y")
            nc.scalar.copy(y_sb[:], ytok_ps[:])
            zb = work.tile([128, QCHUNKS * R], F32, tag="zb", name="zb")
            nc.vector.tensor_add(zb[:], y_sb[:], drep_sb[:])
            z = work.tile([128, QCHUNKS * R], F32, tag="z", name="z")
            nc.scalar.activation(z[:], zb[:], mybir.ActivationFunctionType.Abs)
            zap = work.tile([128, QCHUNKS * R], F32, tag="zap", name="zap")
            for c in range(QCHUNKS):
                m8 = blk.tile([128, 8], F32, tag="m8", name="m8")
                zc = z[:, R * c:R * (c + 1)]
                nc.vector.max(out=m8[:], in_=zc)
                nc.vector.match_replace(
                    out=zap[:, R * c:R * (c + 1)],
                    in_to_replace=m8[:],
                    in_values=zc,
                    imm_value=-1.0,
                )
            mask = zb  # reuse
            nc.vector.tensor_scalar(mask[:], zap[:], 0.0, None, op0=ALU.is_lt)
            act = z  # reuse
            nc.vector.tensor_mul(act[:], y_sb[:], mask[:])
            st[q]["act"] = act

        def emit_backend_pe(q):
            """PE: transpose act to expert-major, fp16 mm2, fp16 stores
            (needs topk(q), which ran during the following quarter's mm1)."""
            act = st[q]["act"]
            pt = tpool.tile([R, TPQ], F32, tag="tp", name="pt")
            for c in range(QCHUNKS):
                nc.tensor.transpose(
                    pt[:, 128 * c:128 * (c + 1)],
                    act[:, R * c:R * (c + 1)],
                    ident[:],
                )
            actT_sb = work.tile([R, TPQ], F16, tag="actT", name="actT")
            nc.vector.tensor_copy(actT_sb[:], pt[:])
            tok0 = TPQ * q
            for c in range(QCHUNKS):
                lhs = actT_sb[:, 128 * c:128 * (c + 1)]
                ot = osb.tile([128, D], F16)
                for n in range(8):
                    ps = opool.tile([128, 512], F32)
                    nc.tensor.matmul(
                        ps[:],
                        lhs,
                        bt2_sb[:, 512 * n:512 * (n + 1)],
                        start=True,
                        stop=True,
                    )
                    osl = ot[:, 512 * n:512 * (n + 1)]
                    if n % 2 == 0:
                        nc.scalar.copy(osl, ps[:])
                    else:
                        nc.vector.tensor_copy(osl, ps[:])
                row0 = tok0 + 128 * c
                nc.gpsimd.dma_start(out=out_d[row0:row0 + 128, :], in_=ot[:])

        # --- software-pipelined emission: PE stream per window is
        # [mm1(q) | ytok(q) | act-trans+mm2(q-1)]; DVE/ACT fill in
        # recombine(q) right after mm1(q) and topk(q) during mm1(q+1). ---
        for q in range(QUARTERS):
            emit_mm1(q)
            emit_front_alu(q)
            if q > 0:
                emit_backend_pe(q - 1)
            emit_ytok_trans(q)
            emit_topk(q)
        emit_backend_pe(QUARTERS - 1)


def _get_nc():
    if "nc" not in _nc_cache:
        _nc_cache["nc"] = _build_kernel()
    return _nc_cache["nc"]


def _pack_a_limbs(hi, lo):
    """[2 limbs, R, D] -> [128, KC*2R] with [p, kc*64+32*l+r] = limb_l[r, 128kc+p]."""
    both = np.stack([hi, lo], axis=0)              # [l, r, D]
    tmp = both.reshape(2, R, KC, 128)              # [l, r, kc, p]
    return np.ascontiguousarray(
        tmp.transpose(3, 2, 0, 1).reshape(128, KC * 2 * R)
    )


def kernel(x, A, B, d, k):
    global LAST_RESULT
    assert int(k) == 8, f"kernel hardcodes k=8, got {k}"
    x = np.asarray(x, dtype=np.float32)
    A = np.asarray(A, dtype=np.float32)
    B = np.asarray(B, dtype=np.float32)
    d = np.asarray(d, dtype=np.float32)
    assert x.shape == (4, 4096, 4096) and A.shape == (R, D) and B.shape == (D, R)

    X = x.reshape(TOKENS, D)
    xh16 = X.astype(np.float16)
    xhT = xh16.T                                   # [D, TOKENS] view
    if USE_LO:
        xl8 = ((X - xh16.astype(np.float32)) * np.float32(2.0 ** XS)).astype(
            ml_dtypes.float8_e4m3)
        xlT = xl8.T

    Ah = A.astype(np.float16)
    Al = (A - Ah.astype(np.float32)).astype(np.float16)
    ATph = _pack_a_limbs(Ah, Al)
    if USE_LO:
        As = A * np.float32(2.0 ** AS)
        A8h = As.astype(ml_dtypes.float8_e4m3)
        A8l = ((As - A8h.astype(np.float32)) * np.float32(2.0 ** AS2)).astype(
            ml_dtypes.float8_e4m3)
        ATpl = _pack_a_limbs(A8h, A8l)

    BT2 = (np.ascontiguousarray(B.T) * np.float32(2.0)).astype(np.float16)
    drep = np.ascontiguousarray(np.tile(d, (128, QCHUNKS)))           # [128, 128]

    nc = _get_nc()
    in_maps = []
    for c in range(N_CORES):
        xhh = np.stack([
            np.ascontiguousarray(
                xhT[:, c * TPC + qq * TPQ: c * TPC + (qq + 1) * TPQ])
            for qq in range(QUARTERS)
        ])
        m = {
            "xh": xhh,
            "ATph": ATph,
            "BT2": BT2,
            "drep": drep,
        }
        if USE_LO:
            m["xl"] = np.stack([
                np.ascontiguousarray(
                    xlT[:, c * TPC + qq * TPQ: c * TPC + (qq + 1) * TPQ])
                for qq in range(QUARTERS)
            ])
            m["ATpl"] = ATpl
        in_maps.append(m)
    trace = bool(int(os.environ.get("KERNEL_TRACE", "0")))
    res = run_bass_kernel_spmd(nc, in_maps, list(range(N_CORES)), trace=trace)
    LAST_RESULT = res
    outs = [res.results[c]["out"] for c in range(N_CORES)]
    full = np.concatenate(outs, axis=0).astype(np.float32)            # [16384, 4096]
    return full.reshape(4, 4096, 4096)
